# revision 11
# baseline (speedup 1.0000x reference)
"""Trainium2 Bass kernel for a Conformer-style MultiHeadedSelfAttentionModule
(Transformer-XL relative-position attention, B=8 S=1024 D=512 H=8).

Distribution: data-parallel over batch. Core b computes batch element b with
the full (replicated) weights; no collectives are needed. The host splits
``inputs`` across the 8 cores and stacks the per-core outputs.

Per-core algorithm (feature-major "transposed" activation layout [D, S]):
  1. LayerNorm in natural layout, PE-transpose -> xnT [D, S] (f32r).
  2. Q/K/P projections with W chunks stationary -> quT/qvT/kT/pT [D, S].
     V projection with xnT chunks stationary -> v natural [S, D] extended with
     a ones column per head (softmax denominator comes out of the attn@v
     matmul for free) and a zeros column (pads the stationary to an even
     free size, an FP32R hardware requirement).
  3. Per head: pos scores computed naturally [q, k], cast to fp16 and written
     to a padded DRAM buffer [S, S+1]; the Transformer-XL rel_shift is a pure
     re-indexing of that buffer's flat layout, read back *transposed* via the
     DMA xbar into [k, q] tiles. Content scores are computed directly in
     [k, q]; score = content + shifted, exp on ACT (max-subtraction-free
     softmax is safe: |score| < 4), then attn@v with v_ext stationary
     accumulates ctx^T [64, q] plus the denominator row.
  4. ctx^T is normalized by the broadcast reciprocal denominators (DRAM-bounce
     partition broadcast) and fed to the output projection with ctx^T chunks
     stationary, giving out [S, D] directly; add bo and DMA out.

All heavy matmuls run as float32r (full-rate fp32, ~1.5e-4 rel err).
"""

import numpy as np

B, S, D, H = 8, 1024, 512, 8
DH = D // H
EPS = 1e-5
P = 128
NCORES = 8

_CACHE = {}


def _build():
    import concourse.bass as bass
    import concourse.bacc as bacc
    import concourse.mybir as mybir
    import concourse.tile as tile
    from concourse.tile import add_dep_helper
    from concourse.masks import make_identity

    F32 = mybir.dt.float32
    F16 = mybir.dt.float16
    F32R = mybir.dt.float32r
    AF = mybir.ActivationFunctionType
    OP = mybir.AluOpType
    AP = bass.AP

    C = 1.0 / float(np.sqrt(np.float32(D)))

    nc = bacc.Bacc("TRN2", target_bir_lowering=False, debug=False,
                   num_devices=NCORES)

    # ---- DRAM I/O ----
    x_d = nc.dram_tensor("x", [S, D], F32, kind="ExternalInput").ap()
    posT_d = nc.dram_tensor("posT", [D, S], F32R, kind="ExternalInput").ap()
    w_d = {}
    for w in ("Wq", "Wk", "Wv", "Wp", "Wo"):
        w_d[w] = nc.dram_tensor(w, [D, D], F32R, kind="ExternalInput").ap()
    vec_d = {}
    for v in ("bq", "bk", "bv", "bo", "u", "vb", "gamma", "beta"):
        vec_d[v] = nc.dram_tensor(v, [D], F32, kind="ExternalInput")
    out_d = nc.dram_tensor("out", [S, D], F32, kind="ExternalOutput").ap()

    # rel-shift scratch (fp16), ping-pong across heads; flat [S*(S+1)]
    pads = [nc.dram_tensor(f"pad{i}", [S * (S + 1)], F16) for i in range(2)]
    dscr = nc.dram_tensor("dscr", [H * S], F32)  # denominator bounce

    def vec_cols(name):
        # DRAM [512] -> SBUF [128, 4] column-chunk layout: (p, m) = v[m*128+p]
        return AP(tensor=vec_d[name], offset=0, ap=[[1, P], [P, D // P]])

    def vec_bcast(name):
        # DRAM [512] broadcast to [128, 512]
        return AP(tensor=vec_d[name], offset=0, ap=[[0, P], [1, D]])

    with tile.TileContext(nc) as tc:
        const = tc.alloc_tile_pool(name="const", bufs=1)
        ident = const.tile([P, P], F32)
        make_identity(nc, ident)
        gamma_b = const.tile([P, D], F32)
        nc.sync.dma_start(out=gamma_b, in_=vec_bcast("gamma"))
        beta_b = const.tile([P, D], F32)
        nc.sync.dma_start(out=beta_b, in_=vec_bcast("beta"))
        bo_b = const.tile([P, D], F32)
        nc.sync.dma_start(out=bo_b, in_=vec_bcast("bo"))
        bv_b = const.tile([P, D], F32)
        nc.sync.dma_start(out=bv_b, in_=vec_bcast("bv"))
        bq_c = const.tile([P, 4], F32)
        nc.sync.dma_start(out=bq_c, in_=vec_cols("bq"))
        bk_c = const.tile([P, 4], F32)
        nc.sync.dma_start(out=bk_c, in_=vec_cols("bk"))
        u_c = const.tile([P, 4], F32)
        nc.sync.dma_start(out=u_c, in_=vec_cols("u"))
        vb_c = const.tile([P, 4], F32)
        nc.sync.dma_start(out=vb_c, in_=vec_cols("vb"))
        bqu_c = const.tile([P, 4], F32)
        nc.vector.tensor_add(out=bqu_c, in0=bq_c, in1=u_c)
        bqv_c = const.tile([P, 4], F32)
        nc.vector.tensor_add(out=bqv_c, in0=bq_c, in1=vb_c)
        eps_t = const.tile([P, 1], F32)
        nc.vector.memset(eps_t, EPS)
        onescols = const.tile([P, 8, 2], F32)
        nc.vector.memset(onescols[:, :, 0:1], 1.0)
        nc.vector.memset(onescols[:, :, 1:2], 0.0)

        # persistent activation tensors (live through attention)
        persist = tc.alloc_tile_pool(name="persist", bufs=1)
        quT = persist.tile([P, 4, S], F32R)
        qvT = persist.tile([P, 4, S], F32R)
        kT = persist.tile([P, 4, S], F32R)
        pT = persist.tile([P, 4, S], F32R)
        v_ext = persist.tile([P, 8, H, DH + 2], F32R)

        # zero column of the PAD buffers (never rewritten afterwards)
        zcol = const.tile([P, 8], F16)
        nc.vector.memset(zcol, 0.0)
        pad_init = []
        for pad in pads:
            pi = nc.sync.dma_start(
                out=AP(tensor=pad, offset=0, ap=[[8 * (S + 1), P], [S + 1, 8]]),
                in_=zcol)
            pad_init.append(pi)

        # ---------------- Phase A-D: LN, transpose, projections --------------
        with tc.tile_pool(name="phAD", bufs=1) as ph, \
             tc.tile_pool(name="phAD_ps", bufs=4, space="PSUM") as psA, \
             tc.tile_pool(name="phAD_stats", bufs=8) as stp:
            x_sb = ph.tile([P, 8, D], F32)
            nc.sync.dma_start(out=x_sb, in_=x_d.rearrange("(t p) d -> p t d", p=P))
            w_sb = {}
            for w in ("Wq", "Wk", "Wv", "Wp"):
                w_sb[w] = ph.tile([P, 4, D], F32R, name=f"wsb_{w}")
                nc.sync.dma_start(out=w_sb[w],
                                  in_=w_d[w].rearrange("(kd p) n -> p kd n", p=P))
            posT_sb = ph.tile([P, 4, S], F32R)
            nc.sync.dma_start(out=posT_sb,
                              in_=posT_d.rearrange("(kd p) n -> p kd n", p=P))
            xnT = ph.tile([P, 4, S], F32R)

            with nc.named_scope("layernorm"):
                for t in range(8):
                    xt = x_sb[:, t, :]
                    st6 = stp.tile([P, 6], F32, tag="st6")
                    nc.vector.bn_stats(out=st6, in_=xt)
                    mv = stp.tile([P, 2], F32, tag="mv")
                    nc.vector.bn_aggr(out=mv, in_=st6)
                    rstd = stp.tile([P, 1], F32, tag="rstd")
                    nc.scalar.activation(out=rstd, in_=mv[:, 1:2], func=AF.Sqrt,
                                         bias=eps_t)
                    nc.vector.reciprocal(out=rstd, in_=rstd)
                    nc.vector.tensor_scalar(out=xt, in0=xt, scalar1=mv[:, 0:1],
                                            scalar2=rstd, op0=OP.subtract,
                                            op1=OP.mult)
                    nc.gpsimd.tensor_mul(out=xt, in0=xt, in1=gamma_b)
                    nc.gpsimd.tensor_add(out=xt, in0=xt, in1=beta_b)

            with nc.named_scope("transpose_x"):
                for t in range(8):
                    for kd in range(4):
                        pt = psA.tile([P, P], F32, tag="tp")
                        nc.tensor.transpose(pt, x_sb[:, t, kd * P:(kd + 1) * P],
                                            ident)
                        nc.vector.tensor_copy(
                            out=xnT[:, kd, t * P:(t + 1) * P], in_=pt)

            with nc.named_scope("proj_qkp"):
                for m in range(4):
                    for n2 in range(2):
                        sl = slice(n2 * 512, (n2 + 1) * 512)
                        pq = psA.tile([P, 512], F32, tag="pj")
                        for kd in range(4):
                            nc.tensor.matmul(pq,
                                             w_sb["Wq"][:, kd, m * P:(m + 1) * P],
                                             xnT[:, kd, sl],
                                             start=(kd == 0), stop=(kd == 3))
                        nc.vector.tensor_scalar(out=quT[:, m, sl], in0=pq,
                                                scalar1=bqu_c[:, m:m + 1],
                                                scalar2=C, op0=OP.add,
                                                op1=OP.mult)
                        nc.vector.tensor_scalar(out=qvT[:, m, sl], in0=pq,
                                                scalar1=bqv_c[:, m:m + 1],
                                                scalar2=C, op0=OP.add,
                                                op1=OP.mult)
                        pk = psA.tile([P, 512], F32, tag="pj")
                        for kd in range(4):
                            nc.tensor.matmul(pk,
                                             w_sb["Wk"][:, kd, m * P:(m + 1) * P],
                                             xnT[:, kd, sl],
                                             start=(kd == 0), stop=(kd == 3))
                        nc.vector.tensor_scalar(out=kT[:, m, sl], in0=pk,
                                                scalar1=bk_c[:, m:m + 1],
                                                scalar2=None, op0=OP.add)
                        pp = psA.tile([P, 512], F32, tag="pj")
                        for kd in range(4):
                            nc.tensor.matmul(pp,
                                             w_sb["Wp"][:, kd, m * P:(m + 1) * P],
                                             posT_sb[:, kd, sl],
                                             start=(kd == 0), stop=(kd == 3))
                        nc.vector.tensor_copy(out=pT[:, m, sl], in_=pp)

            with nc.named_scope("proj_v"):
                for sd in range(8):
                    pv = psA.tile([P, 512], F32, tag="pj")
                    for kd in range(4):
                        nc.tensor.matmul(pv,
                                         xnT[:, kd, sd * P:(sd + 1) * P],
                                         w_sb["Wv"][:, kd, :],
                                         start=(kd == 0), stop=(kd == 3))
                    nc.vector.tensor_add(
                        out=v_ext[:, sd, :, 0:DH],
                        in0=pv.rearrange("p (h c) -> p h c", h=H),
                        in1=bv_b.rearrange("p (h c) -> p h c", h=H))
                for sd in range(8):
                    nc.vector.tensor_copy(out=v_ext[:, sd, :, DH:DH + 2],
                                          in_=onescols)

        # ---------------- Phase E: attention ---------------------------------
        attn = tc.alloc_tile_pool(name="attn", bufs=1)
        ctxT = attn.tile([P, 4, S], F32R)
        wo_sb = attn.tile([P, 4, D], F32R)
        nc.sync.dma_start(out=wo_sb,
                          in_=w_d["Wo"].rearrange("(kd p) n -> p kd n", p=P))

        prev_reads = [list(pad_init[0:1]), list(pad_init[1:2])]
        with tc.tile_pool(name="p16", bufs=3) as p16p, \
             tc.tile_pool(name="sh16", bufs=3) as shp, \
             tc.tile_pool(name="scoreb", bufs=4) as scp, \
             tc.tile_pool(name="expb", bufs=4) as exp_p, \
             tc.tile_pool(name="recipb", bufs=2) as rbp, \
             tc.tile_pool(name="ps_pos", bufs=2, space="PSUM") as ps_pos, \
             tc.tile_pool(name="ps_cont", bufs=2, space="PSUM") as ps_cont, \
             tc.tile_pool(name="ps_ctx", bufs=4, space="PSUM") as ps_ctx:
            for h in range(H):
                with nc.named_scope(f"head{h}"):
                    pad = pads[h % 2]
                    kd_h, off = h // 2, (h % 2) * DH

                    # pos scores natural [q, k] -> fp16 -> PAD rows
                    wr = []
                    for qa in range(8):
                        p16 = p16p.tile([P, S], F16, tag="p16")
                        for ka in range(2):
                            pp = ps_pos.tile([P, 512], F32, tag="pos")
                            nc.tensor.matmul(
                                pp,
                                qvT[off:off + DH, kd_h, qa * P:(qa + 1) * P],
                                pT[off:off + DH, kd_h, ka * 512:(ka + 1) * 512],
                                start=True, stop=True)
                            nc.scalar.activation(
                                out=p16[:, ka * 512:(ka + 1) * 512], in_=pp,
                                func=AF.Copy)
                        wi = nc.sync.dma_start(
                            out=AP(tensor=pad, offset=qa * P * (S + 1) + 1,
                                   ap=[[S + 1, P], [1, S]]),
                            in_=p16)
                        for r in prev_reads[h % 2]:
                            add_dep_helper(wi.ins, r.ins)
                        wr.append(wi)

                    # shifted-transposed read + content + softmax + attn@v
                    rds = []
                    pctx = [ps_ctx.tile([DH + 2, 512], F32, tag="ctx", name=f"pctx{h}_{i}")
                            for i in range(2)]
                    for ka2 in range(8):
                        sh = shp.tile([P, S], F16, tag="sh")
                        ri = nc.sync.dma_start(
                            out=sh,
                            in_=AP(tensor=pad, offset=S + ka2 * P,
                                   ap=[[S, S], [1, P]]),
                            transpose=True)
                        for w in wr:
                            add_dep_helper(ri.ins, w.ins)
                        rds.append(ri)
                        for qa2 in range(2):
                            qsl = slice(qa2 * 512, (qa2 + 1) * 512)
                            pc = ps_cont.tile([P, 512], F32, tag="cont")
                            nc.tensor.matmul(
                                pc,
                                kT[off:off + DH, kd_h, ka2 * P:(ka2 + 1) * P],
                                quT[off:off + DH, kd_h, qsl],
                                start=True, stop=True)
                            sc = scp.tile([P, 512], F32, tag="sc")
                            nc.vector.tensor_add(out=sc, in0=pc, in1=sh[:, qsl])
                            ex = exp_p.tile([P, 512], F32R, tag="ex")
                            nc.scalar.activation(out=ex, in_=sc, func=AF.Exp)
                            nc.tensor.matmul(pctx[qa2],
                                             v_ext[:, ka2, h, :], ex,
                                             start=(ka2 == 0), stop=(ka2 == 7))
                    prev_reads[h % 2] = rds

                    # normalize ctx^T by softmax denominators
                    dtmp = rbp.tile([1, S], F32, tag="dtmp")
                    for qa2 in range(2):
                        nc.vector.tensor_copy(
                            out=dtmp[:, qa2 * 512:(qa2 + 1) * 512],
                            in_=pctx[qa2][DH:DH + 1, :])
                    db = nc.sync.dma_start(
                        out=AP(tensor=dscr, offset=h * S, ap=[[1, S]]),
                        in_=dtmp)
                    rb = rbp.tile([DH, S], F32, tag="rb")
                    rbi = nc.sync.dma_start(
                        out=rb,
                        in_=AP(tensor=dscr, offset=h * S, ap=[[0, DH], [1, S]]))
                    add_dep_helper(rbi.ins, db.ins)
                    nc.vector.reciprocal(out=rb, in_=rb)
                    for qa2 in range(2):
                        qsl = slice(qa2 * 512, (qa2 + 1) * 512)
                        nc.vector.tensor_mul(
                            out=ctxT[off:off + DH, kd_h, qsl],
                            in0=pctx[qa2][0:DH, :], in1=rb[:, qsl])

        # ---------------- Phase F: output projection --------------------------
        with nc.named_scope("out_proj"), \
             tc.tile_pool(name="outb", bufs=3) as op, \
             tc.tile_pool(name="ps_out", bufs=2, space="PSUM") as ps_out:
            for sd in range(8):
                po = ps_out.tile([P, 512], F32, tag="po")
                for kd in range(4):
                    nc.tensor.matmul(po, ctxT[:, kd, sd * P:(sd + 1) * P],
                                     wo_sb[:, kd, :],
                                     start=(kd == 0), stop=(kd == 3))
                ot = op.tile([P, D], F32, tag="ot")
                nc.vector.tensor_add(out=ot, in0=po, in1=bo_b)
                nc.sync.dma_start(out=out_d[sd * P:(sd + 1) * P, :], in_=ot)

        attn.release()
        persist.release()
        const.release()

    nc.compile()
    return nc


def get_nc():
    if "nc" not in _CACHE:
        _CACHE["nc"] = _build()
    return _CACHE["nc"]


def make_in_maps(inputs):
    x = np.ascontiguousarray(np.asarray(inputs["inputs"], dtype=np.float32))
    posT = np.ascontiguousarray(
        np.asarray(inputs["pos_emb"], dtype=np.float32).T)
    shared = {
        "posT": posT,
        "Wq": np.ascontiguousarray(np.asarray(inputs["Wq"], np.float32)),
        "Wk": np.ascontiguousarray(np.asarray(inputs["Wk"], np.float32)),
        "Wv": np.ascontiguousarray(np.asarray(inputs["Wv"], np.float32)),
        "Wp": np.ascontiguousarray(np.asarray(inputs["Wp"], np.float32)),
        "Wo": np.ascontiguousarray(np.asarray(inputs["Wo"], np.float32)),
        "bq": np.asarray(inputs["bq"], np.float32),
        "bk": np.asarray(inputs["bk"], np.float32),
        "bv": np.asarray(inputs["bv"], np.float32),
        "bo": np.asarray(inputs["bo"], np.float32),
        "u": np.ascontiguousarray(
            np.asarray(inputs["u_bias"], np.float32).reshape(D)),
        "vb": np.ascontiguousarray(
            np.asarray(inputs["v_bias"], np.float32).reshape(D)),
        "gamma": np.asarray(inputs["gamma"], np.float32),
        "beta": np.asarray(inputs["beta"], np.float32),
    }
    return [dict(shared, x=np.ascontiguousarray(x[b])) for b in range(NCORES)]


def kernel(**inputs):
    from concourse import bass_utils

    nc = get_nc()
    in_maps = make_in_maps(inputs)
    res = bass_utils.run_bass_kernel_spmd(nc, in_maps,
                                          core_ids=list(range(NCORES)))
    out = np.stack([res.results[b]["out"] for b in range(NCORES)], axis=0)
    return out.astype(np.float32)
